# revision 7
# baseline (speedup 1.0000x reference)
"""Trainium2 Bass kernel for DRM topk-masking matching module.

reference semantics (per (b, h) item):
  news_mean = news_embedding.mean(axis=-2)                       # [B,H,S,D]
  scores    = cosine(news_mean[b,h,s,:], user_repr[b])           # [B,H,S]
  score_k, score_kid = top_k(scores, 10)                         # [B,H,10]
  out = news_embedding[b,h,score_kid,:,:] * score_k[...,None,None]
returns (out [B,H,10,L,D] f32, score_kid [B,H,10] int32)

Sharding: pure data parallel on batch: 16 batches -> 8 cores x 2 batches.
"""

import numpy as np

import concourse.bass as bass
import concourse.tile as tile
from concourse import bacc, mybir
from concourse import bass_utils
from concourse.masks import make_identity

# problem shape (hardcoded; kernel.py must be self-contained)
B, H, S, L, D = 16, 50, 128, 2, 128
K = 10
NCORES = 8
BPC = B // NCORES          # batches per core = 2
ITEMS = BPC * H            # items per core = 100
CHUNK = 10                 # items per load chunk
NCHUNK = ITEMS // CHUNK    # 10
LD = L * D                 # 256
F32 = mybir.dt.float32
U32 = mybir.dt.uint32
I32 = mybir.dt.int32

NEG_BIG = -3.0e38          # replacement value for match_replace (< any cosine)


def _build_kernel(reps=1):
    nc = bacc.Bacc("TRN2", target_bir_lowering=False, debug=False,
                   enable_asserts=False)

    news = nc.dram_tensor("news", [BPC, H, S, L, D], F32, kind="ExternalInput").ap()
    user = nc.dram_tensor("user", [BPC, 1, D], F32, kind="ExternalInput").ap()
    wps = nc.dram_tensor("wps", [BPC, H, K, L, D], F32, kind="ExternalOutput").ap()
    kid = nc.dram_tensor("kid", [BPC, H, K], I32, kind="ExternalOutput").ap()

    # views
    # chunk load: iterate (s, item, l, d) so src/dst stream orders match
    news_sild = news.rearrange("b h s l d -> s (b h) l d")     # [S, ITEMS, L, D]
    news_flat = news.rearrange("b h s l d -> (b h s) (l d)")   # [ITEMS*S, LD]
    wps_flat = wps.rearrange("b h k l d -> (b h) (k l d)")     # [ITEMS, K*LD]
    kid_flat = kid.rearrange("b h k -> (b h) k")               # [ITEMS, K]

    with tile.TileContext(nc) as tc:
        with (
            tc.tile_pool(name="const", bufs=1) as const_pool,
            tc.tile_pool(name="io", bufs=3) as io_pool,
            tc.tile_pool(name="work", bufs=2) as work_pool,
            tc.tile_pool(name="acc", bufs=1) as acc_pool,
            tc.tile_pool(name="psum", bufs=1, space="PSUM") as psum_pool,
        ):
            # ---- constants / user vector setup ----
            identity = const_pool.tile([128, 128], F32)
            make_identity(nc, identity[:])

            # normalized user vector per batch, broadcast to 128 partitions
            userb = const_pool.tile([128, BPC, D], F32)
            for b in range(BPC):
                u_raw = const_pool.tile([1, D], F32, tag="u_raw")
                nc.sync.dma_start(out=u_raw[:], in_=user[b, :, :])
                u_sq = const_pool.tile([1, 1], F32, tag="u_sq")
                u_scr = const_pool.tile([1, D], F32, tag="u_scr")
                nc.scalar.activation(u_scr[:], u_raw[:],
                                     mybir.ActivationFunctionType.Square)
                nc.vector.tensor_reduce(out=u_sq[:], in_=u_scr[:],
                                        axis=mybir.AxisListType.X,
                                        op=mybir.AluOpType.add)
                # max(||u||, eps)^2 ; eps = 1e-12
                nc.vector.tensor_scalar_max(u_sq[:], u_sq[:], 1e-24)
                u_rcp = const_pool.tile([1, 1], F32, tag="u_rcp")
                nc.vector.reciprocal(u_rcp[:], u_sq[:])
                u_rsd = const_pool.tile([1, 1], F32, tag="u_rsd")
                nc.scalar.activation(u_rsd[:], u_rcp[:],
                                     mybir.ActivationFunctionType.Sqrt)
                u_n = const_pool.tile([1, D], F32, tag="u_n")
                nc.vector.tensor_scalar_mul(u_n[:], u_raw[:], u_rsd[:])
                nc.gpsimd.partition_broadcast(userb[:, b, :], u_n[:])

            def body():
                # per-s numerator / squared-norm, one column per item
                num_all = acc_pool.tile([128, ITEMS], F32, tag="num_all")
                den_all = acc_pool.tile([128, ITEMS], F32, tag="den_all")

                # ---- main loop: load + reduce scores ----
                for c in range(NCHUNK):
                    b = (c * CHUNK) // H  # batch of all items in this chunk
                    chunk = io_pool.tile([128, CHUNK, L, D], F32, tag="chunk")
                    nc.sync.dma_start(
                        out=chunk[:],
                        in_=news_sild[:, c * CHUNK:(c + 1) * CHUNK, :, :],
                    )
                    # level sum S = A + B (gpsimd to offload DVE)
                    s_t = work_pool.tile([128, CHUNK, D], F32, tag="s_t")
                    nc.gpsimd.tensor_tensor(
                        out=s_t[:], in0=chunk[:, :, 0, :], in1=chunk[:, :, 1, :],
                        op=mybir.AluOpType.add,
                    )
                    # squared norm via ACT square + DVE reduce
                    sq_t = work_pool.tile([128, CHUNK, D], F32, tag="sq_t")
                    nc.scalar.activation(sq_t[:], s_t[:],
                                         mybir.ActivationFunctionType.Square)
                    nc.vector.tensor_reduce(
                        out=den_all[:, c * CHUNK:(c + 1) * CHUNK], in_=sq_t[:],
                        axis=mybir.AxisListType.X, op=mybir.AluOpType.add,
                    )
                    # numerator: mul by broadcast user vector, reduce over d
                    p_t = work_pool.tile([128, CHUNK, D], F32, tag="p_t")
                    nc.vector.tensor_tensor(
                        out=p_t[:], in0=s_t[:],
                        in1=userb[:, b, :].unsqueeze(1).broadcast_to(
                            [128, CHUNK, D]),
                        op=mybir.AluOpType.mult,
                    )
                    nc.vector.tensor_reduce(
                        out=num_all[:, c * CHUNK:(c + 1) * CHUNK], in_=p_t[:],
                        axis=mybir.AxisListType.X, op=mybir.AluOpType.add,
                    )

                # ---- scores = num * rsqrt(den) ; [s, item] -> [item, s] ----
                recip = acc_pool.tile([128, ITEMS], F32, tag="recip")
                nc.vector.reciprocal(recip[:], den_all[:])
                rsd = acc_pool.tile([128, ITEMS], F32, tag="rsd")
                nc.scalar.activation(rsd[:], recip[:],
                                     mybir.ActivationFunctionType.Sqrt)
                scoresT = acc_pool.tile([128, ITEMS], F32, tag="scoresT")
                nc.vector.tensor_tensor(out=scoresT[:], in0=num_all[:],
                                        in1=rsd[:], op=mybir.AluOpType.mult)
                ps_scores = psum_pool.tile([ITEMS, 128], F32, tag="ps_scores")
                nc.tensor.transpose(ps_scores[:], scoresT[:], identity[:])
                scores = acc_pool.tile([ITEMS, 128], F32, tag="scores")
                nc.vector.tensor_copy(scores[:], ps_scores[:])

                # ---- top-10 via two rounds of max8 ----
                m1 = acc_pool.tile([ITEMS, 8], F32, tag="m1")
                i1 = acc_pool.tile([ITEMS, 8], U32, tag="i1")
                nc.vector.max(out=m1[:], in_=scores[:])
                nc.vector.max_index(out=i1[:], in_max=m1[:], in_values=scores[:])
                sc2 = acc_pool.tile([ITEMS, 128], F32, tag="sc2")
                nc.vector.match_replace(out=sc2[:], in_to_replace=m1[:],
                                        in_values=scores[:], imm_value=NEG_BIG)
                m2 = acc_pool.tile([ITEMS, 8], F32, tag="m2")
                i2 = acc_pool.tile([ITEMS, 8], U32, tag="i2")
                nc.vector.max(out=m2[:], in_=sc2[:])
                nc.vector.max_index(out=i2[:], in_max=m2[:], in_values=sc2[:])

                score_k = acc_pool.tile([ITEMS, K], F32, tag="score_k")
                nc.vector.tensor_copy(score_k[:, 0:8], m1[:])
                nc.vector.tensor_copy(score_k[:, 8:K], m2[:, 0:2])
                kid_t = acc_pool.tile([ITEMS, K], U32, tag="kid_t")
                nc.vector.tensor_copy(kid_t[:, 0:8], i1[:])
                nc.vector.tensor_copy(kid_t[:, 8:K], i2[:, 0:2])

                # ---- gather selected rows from HBM ----
                iota_t = acc_pool.tile([ITEMS, K], U32, tag="iota_t")
                nc.gpsimd.iota(iota_t[:], pattern=[[0, K]], base=0,
                               channel_multiplier=S)
                idx_g = acc_pool.tile([ITEMS, K], U32, tag="idx_g")
                nc.vector.tensor_tensor(out=idx_g[:], in0=kid_t[:],
                                        in1=iota_t[:], op=mybir.AluOpType.add)
                g_all = acc_pool.tile([ITEMS, K, LD], F32, tag="g_all")
                # NOTE: one offset per partition per call — multi-offset
                # indirect DMA returns garbage on HW (works only in CoreSim).
                for k in range(K):
                    nc.gpsimd.indirect_dma_start(
                        out=g_all[:, k, :], out_offset=None,
                        in_=news_flat[:],
                        in_offset=bass.IndirectOffsetOnAxis(
                            ap=idx_g[:, k:k + 1], axis=0),
                    )

                # ---- weight by score and store ----
                for k in range(K):
                    nc.vector.tensor_scalar_mul(g_all[:, k, :], g_all[:, k, :],
                                                score_k[:, k:k + 1])
                nc.sync.dma_start(out=wps_flat[:], in_=g_all[:])
                nc.sync.dma_start(out=kid_flat[:], in_=kid_t[:].bitcast(I32))

            for _rep in range(reps):
                body()

    nc.compile()
    return nc


_NC_CACHE = None


def _get_nc():
    global _NC_CACHE
    if _NC_CACHE is None:
        _NC_CACHE = _build_kernel()
    return _NC_CACHE


def kernel(news_embedding, user_repr):
    news = np.ascontiguousarray(np.asarray(news_embedding, dtype=np.float32))
    user = np.ascontiguousarray(np.asarray(user_repr, dtype=np.float32))
    assert news.shape == (B, H, S, L, D), news.shape
    assert user.shape == (B, 1, D), user.shape

    nc = _get_nc()
    in_maps = []
    for i in range(NCORES):
        sl = slice(i * BPC, (i + 1) * BPC)
        in_maps.append({
            "news": np.ascontiguousarray(news[sl]),
            "user": np.ascontiguousarray(user[sl]),
        })
    res = bass_utils.run_bass_kernel_spmd(nc, in_maps,
                                          core_ids=list(range(NCORES)))
    wps = np.concatenate([r["wps"] for r in res.results], axis=0)
    kid = np.concatenate([r["kid"] for r in res.results], axis=0)
    return wps, kid


# revision 29
# speedup vs baseline: 193.7953x; 193.7953x over previous
"""Trainium2 Bass kernel for DRM topk-masking matching module.

reference semantics (per (b, h) item):
  news_mean = news_embedding.mean(axis=-2)                       # [B,H,S,D]
  scores    = cosine(news_mean[b,h,s,:], user_repr[b])           # [B,H,S]
  score_k, score_kid = top_k(scores, 10)                         # [B,H,10]
  out = news_embedding[b,h,score_kid,:,:] * score_k[...,None,None]
returns (out [B,H,10,L,D] f32, score_kid [B,H,10] int32)

Sharding: pure data parallel on batch: 16 batches -> 8 cores x 2 batches.
"""

import numpy as np

import concourse.bass as bass
import concourse.tile as tile
from concourse import bacc, mybir
from concourse import bass_utils
from concourse.masks import make_identity

# problem shape (hardcoded; kernel.py must be self-contained)
B, H, S, L, D = 16, 50, 128, 2, 128
K = 10
NCORES = 8
BPC = B // NCORES          # batches per core = 2
ITEMS = BPC * H            # items per core = 100
CHUNK = 10                 # items per load chunk
NCHUNK = ITEMS // CHUNK    # 10
LD = L * D                 # 256
F32 = mybir.dt.float32
U32 = mybir.dt.uint32
I32 = mybir.dt.int32

NEG_BIG = -3.0e38          # replacement value for match_replace (< any cosine)


def _pool_avg_on(nc, eng, out_ap, in_ap):
    """AvgPool over the innermost dim, on a chosen engine (DVE or Pool).

    Average (not sum) keeps the reduction exact up to a power-of-two scale,
    which we compensate exactly later. InstPool is in the gpsimd `standard`
    library, so it can run on the Pool engine for load balancing.
    """
    # 5-dim AP with the singleton dims at positions 2,3 — the only
    # arrangement that passes walrus's is_valid_s4d4_pl_addr check.
    while len(in_ap.shape) < 5:
        in_ap = in_ap.unsqueeze(2)
    in_pap = eng.lower_ap(in_ap, opt=False)
    return eng.add_instruction(
        mybir.InstPool(
            name=f"I-{nc.next_id()}",
            func=mybir.PoolFunctionType.avg,
            ins=[in_pap],
            outs=[eng.lower_ap(out_ap)],
        )
    )


def _build_kernel(reps=1):
    nc = bacc.Bacc("TRN2", target_bir_lowering=False, debug=False,
                   enable_asserts=False)

    news = nc.dram_tensor("news", [BPC, H, S, L, D], F32, kind="ExternalInput").ap()
    user = nc.dram_tensor("user", [BPC, 1, D], F32, kind="ExternalInput").ap()
    wps = nc.dram_tensor("wps", [BPC, H, K, L, D], F32, kind="ExternalOutput").ap()
    kid = nc.dram_tensor("kid", [BPC, H, K], I32, kind="ExternalOutput").ap()

    # views
    # chunk load: iterate (s, item, l, d) so src/dst stream orders match
    news_sild = news.rearrange("b h s l d -> s (b h) l d")     # [S, ITEMS, L, D]
    news_flat = news.rearrange("b h s l d -> (b h s) (l d)")   # [ITEMS*S, LD]
    wps_flat = wps.rearrange("b h k l d -> (b h) (k l d)")     # [ITEMS, K*LD]
    kid_flat = kid.rearrange("b h k -> (b h) k")               # [ITEMS, K]

    with tile.TileContext(nc) as tc:
        with (
            tc.tile_pool(name="const", bufs=1) as const_pool,
            tc.tile_pool(name="io", bufs=4) as io_pool,
            tc.tile_pool(name="work", bufs=3) as work_pool,
            tc.tile_pool(name="acc", bufs=1) as acc_pool,
            tc.tile_pool(name="psum", bufs=1, space="PSUM") as psum_pool,
        ):
            # ---- constants / user vector setup ----
            identity = const_pool.tile([128, 128], F32)
            make_identity(nc, identity[:])

            # normalized user vector per batch, broadcast to 128 partitions
            userb = const_pool.tile([128, BPC, D], F32)
            for b in range(BPC):
                u_raw = const_pool.tile([1, D], F32, tag="u_raw")
                nc.sync.dma_start(out=u_raw[:], in_=user[b, :, :])
                u_sq = const_pool.tile([1, 1], F32, tag="u_sq")
                u_scr = const_pool.tile([1, D], F32, tag="u_scr")
                nc.scalar.activation(u_scr[:], u_raw[:],
                                     mybir.ActivationFunctionType.Square)
                nc.vector.tensor_reduce(out=u_sq[:], in_=u_scr[:],
                                        axis=mybir.AxisListType.X,
                                        op=mybir.AluOpType.add)
                # max(||u||, eps)^2 ; eps = 1e-12
                nc.vector.tensor_scalar_max(u_sq[:], u_sq[:], 1e-24)
                u_rcp = const_pool.tile([1, 1], F32, tag="u_rcp")
                nc.vector.reciprocal(u_rcp[:], u_sq[:])
                u_rsd = const_pool.tile([1, 1], F32, tag="u_rsd")
                nc.scalar.activation(u_rsd[:], u_rcp[:],
                                     mybir.ActivationFunctionType.Sqrt)
                u_n = const_pool.tile([1, D], F32, tag="u_n")
                nc.vector.tensor_scalar_mul(u_n[:], u_raw[:], u_rsd[:])
                nc.gpsimd.partition_broadcast(userb[:, b, :], u_n[:])

            def body():
                # per-s numerator / squared-norm, one column per item
                num_all = acc_pool.tile([128, ITEMS], F32, tag="num_all")
                den_all = acc_pool.tile([128, ITEMS], F32, tag="den_all")

                # ---- main loop: load + reduce scores ----
                # Engine split (cost-model balanced): gpsimd does the
                # level-sum (+ a few den-folds), DVE the user-mul, folds and
                # avg-pools, ACT the squares. Last two chunks are half-size to
                # shorten the end-of-loop drain into the topk/gather tail.
                sizes = [CHUNK] * (NCHUNK - 1) + [CHUNK // 2, CHUNK // 2]
                starts = [sum(sizes[:i]) for i in range(len(sizes))]
                for c, (i0, cn) in enumerate(zip(starts, sizes)):
                    b = i0 // H  # batch of all items in this chunk
                    chunk = io_pool.tile([128, CHUNK, L, D], F32, tag="chunk")
                    chunk = chunk[:, 0:cn, :, :]
                    nc.sync.dma_start(
                        out=chunk[:],
                        in_=news_sild[:, i0:i0 + cn, :, :],
                    )
                    # level sum S = A + B (gpsimd, to offload DVE)
                    s_t = work_pool.tile([128, CHUNK, D], F32, tag="s_t")
                    s_t = s_t[:, 0:cn, :]
                    nc.gpsimd.tensor_tensor(
                        out=s_t[:], in0=chunk[:, :, 0, :], in1=chunk[:, :, 1, :],
                        op=mybir.AluOpType.add,
                    )
                    # squared norm: ACT square, fold 128->64 (2-input add,
                    # engine chosen for balance), then DVE avg-pool over 64
                    sq_t = work_pool.tile([128, CHUNK, D], F32, tag="sq_t")
                    sq_t = sq_t[:, 0:cn, :]
                    nc.scalar.activation(sq_t[:], s_t[:],
                                         mybir.ActivationFunctionType.Square)
                    sqf = work_pool.tile([128, CHUNK, D // 2], F32, tag="sqf")
                    sqf = sqf[:, 0:cn, :]
                    den_eng = nc.gpsimd if c in (2, 5, 8) else nc.vector
                    den_eng.tensor_tensor(
                        out=sqf[:], in0=sq_t[:, :, 0:D // 2],
                        in1=sq_t[:, :, D // 2:D], op=mybir.AluOpType.add,
                    )
                    _pool_avg_on(nc, nc.vector,
                                 den_all[:, i0:i0 + cn], sqf[:])
                    # numerator: mul by broadcast user vector, fold, avg-pool
                    p_t = work_pool.tile([128, CHUNK, D], F32, tag="p_t")
                    p_t = p_t[:, 0:cn, :]
                    nc.vector.tensor_tensor(
                        out=p_t[:], in0=s_t[:],
                        in1=userb[:, b, :].unsqueeze(1).broadcast_to(
                            [128, cn, D]),
                        op=mybir.AluOpType.mult,
                    )
                    pf = work_pool.tile([128, CHUNK, D // 2], F32, tag="pf")
                    pf = pf[:, 0:cn, :]
                    nc.vector.tensor_tensor(
                        out=pf[:], in0=p_t[:, :, 0:D // 2],
                        in1=p_t[:, :, D // 2:D], op=mybir.AluOpType.add,
                    )
                    _pool_avg_on(nc, nc.vector,
                                 num_all[:, i0:i0 + cn], pf[:])

                # ---- scores: num_avg64 * rsqrt(den) = score/64 (uniform) ----
                # den_avg64 = den/64; recip*2^-6 (exact, folded into ACT scale)
                # -> 1/den; sqrt -> rsqrt(den)
                recip = acc_pool.tile([128, ITEMS], F32, tag="recip")
                nc.vector.reciprocal(recip[:], den_all[:])
                rsd = acc_pool.tile([128, ITEMS], F32, tag="rsd")
                nc.scalar.activation(rsd[:], recip[:],
                                     mybir.ActivationFunctionType.Sqrt,
                                     scale=1.0 / 64.0)
                scoresT = acc_pool.tile([128, ITEMS], F32, tag="scoresT")
                nc.vector.tensor_tensor(out=scoresT[:], in0=num_all[:],
                                        in1=rsd[:], op=mybir.AluOpType.mult)
                ps_scores = psum_pool.tile([ITEMS, 128], F32, tag="ps_scores")
                nc.tensor.transpose(ps_scores[:], scoresT[:], identity[:])
                scores = acc_pool.tile([ITEMS, 128], F32, tag="scores")
                nc.scalar.copy(scores[:], ps_scores[:])

                # ---- top-10 via two rounds of max8 ----
                m1 = acc_pool.tile([ITEMS, 8], F32, tag="m1")
                i1 = acc_pool.tile([ITEMS, 8], U32, tag="i1")
                nc.vector.max(out=m1[:], in_=scores[:])
                nc.vector.max_index(out=i1[:], in_max=m1[:], in_values=scores[:])
                sc2 = acc_pool.tile([ITEMS, 128], F32, tag="sc2")
                nc.vector.match_replace(out=sc2[:], in_to_replace=m1[:],
                                        in_values=scores[:], imm_value=NEG_BIG)
                m2 = acc_pool.tile([ITEMS, 8], F32, tag="m2")
                i2 = acc_pool.tile([ITEMS, 8], U32, tag="i2")
                nc.vector.max(out=m2[:], in_=sc2[:])
                nc.vector.max_index(out=i2[:], in_max=m2[:], in_values=sc2[:])

                # top-k values are score/64 — scale back exactly (x64)
                score_k = acc_pool.tile([ITEMS, K], F32, tag="score_k")
                nc.scalar.mul(score_k[:, 0:8], m1[:], 64.0)
                nc.scalar.mul(score_k[:, 8:K], m2[:, 0:2], 64.0)
                kid_t = acc_pool.tile([ITEMS, K], U32, tag="kid_t")
                nc.vector.tensor_copy(kid_t[:, 0:8], i1[:])
                nc.vector.tensor_copy(kid_t[:, 8:K], i2[:, 0:2])
                nc.sync.dma_start(out=kid_flat[:], in_=kid_t[:].bitcast(I32))

                # ---- gather selected rows from HBM ----
                iota_t = acc_pool.tile([ITEMS, K], U32, tag="iota_t")
                nc.gpsimd.iota(iota_t[:], pattern=[[0, K]], base=0,
                               channel_multiplier=S)
                idx_g = acc_pool.tile([ITEMS, K], U32, tag="idx_g")
                nc.vector.tensor_tensor(out=idx_g[:], in0=kid_t[:],
                                        in1=iota_t[:], op=mybir.AluOpType.add)
                g_all = acc_pool.tile([ITEMS, K, LD], F32, tag="g_all")
                # NOTE: one offset per partition per call — multi-offset
                # indirect DMA returns garbage on HW (works only in CoreSim).
                # Weight-mul and store per k so they overlap later gathers.
                wps_view = wps_flat.rearrange("i (k e) -> i k e", k=K)
                for k in range(K):
                    nc.gpsimd.indirect_dma_start(
                        out=g_all[:, k, :], out_offset=None,
                        in_=news_flat[:],
                        in_offset=bass.IndirectOffsetOnAxis(
                            ap=idx_g[:, k:k + 1], axis=0),
                    )
                    nc.vector.tensor_scalar_mul(g_all[:, k, :], g_all[:, k, :],
                                                score_k[:, k:k + 1])
                    nc.sync.dma_start(out=wps_view[:, k, :],
                                      in_=g_all[:, k, :])

            for _rep in range(reps):
                body()

    nc.compile()
    return nc


_NC_CACHE = None


def _get_nc():
    global _NC_CACHE
    if _NC_CACHE is None:
        _NC_CACHE = _build_kernel()
    return _NC_CACHE


def kernel(news_embedding, user_repr):
    news = np.ascontiguousarray(np.asarray(news_embedding, dtype=np.float32))
    user = np.ascontiguousarray(np.asarray(user_repr, dtype=np.float32))
    assert news.shape == (B, H, S, L, D), news.shape
    assert user.shape == (B, 1, D), user.shape

    nc = _get_nc()
    in_maps = []
    for i in range(NCORES):
        sl = slice(i * BPC, (i + 1) * BPC)
        in_maps.append({
            "news": np.ascontiguousarray(news[sl]),
            "user": np.ascontiguousarray(user[sl]),
        })
    res = bass_utils.run_bass_kernel_spmd(nc, in_maps,
                                          core_ids=list(range(NCORES)))
    wps = np.concatenate([r["wps"] for r in res.results], axis=0)
    kid = np.concatenate([r["kid"] for r in res.results], axis=0)
    return wps, kid
